# revision 8
# baseline (speedup 1.0000x reference)
"""Trainium2 Bass kernel for nn_KroneckerLayer.

out[n] = theta1 * kron[n] + mean_n(theta2 * kron) + theta3 * mean_k kron[nbr(n,k)]
with kron[n] = outer(x[n] in R^8, v[n] in R^256), output (8192, 64, 32) f32.

Strategy: shard nodes across 8 cores (1024 each). Per 14-node tile, one
indirect-DMA gather pulls the 126 needed table rows (self + 8 neighbors,
row = [v | x] concat, 264 f32) into SBUF; DVE scales rows by theta
patterns, a mask multiply builds a block-diagonal lhsT, and a single PE
matmul emits all 14 nodes' 2048 outputs into PSUM. The global-mean term
is 8 chunk matmuls over the local shard + an 8KB AllReduce, folded into
the PSUM->SBUF eviction add.
"""

import sys
import types

for _p in ("/opt/trn_rl_repo", "/root/.axon_site/_ro/trn_rl_repo"):
    if _p not in sys.path:
        sys.path.insert(0, _p)

import numpy as np

import concourse.bass as bass
import concourse.bacc as bacc
import concourse.tile as tile
import concourse.mybir as mybir
from concourse.bass import IndirectOffsetOnAxis
from concourse.bass_interp import get_hw_module
import concourse.bass_utils as bass_utils
from concourse.bass_utils import run_bass_kernel_spmd

# Avoid S3 artifact upload during tracing (no creds in container)
bass_utils.upload_artifacts = lambda tmpdir: "local://" + tmpdir

N, M, D, C, K = 8192, 8, 8, 32, 8
R = D * C              # 256, v row
TW = R + M             # 264, table row [v | x]
NCORES = 8
NL = N // NCORES       # 1024 local nodes per core
BT = 14                # nodes per tile
EPN = K + 1            # 9 edges per node (self + K neighbors)
NFULL = NL // BT       # 73 full tiles
REM = NL - NFULL * BT  # 2 nodes in the last tile
NT = NFULL + (1 if REM else 0)  # 74
OUTW = M * R           # 2048 floats per node

F32 = mybir.dt.float32
I32 = mybir.dt.int32


def _ensure_profile_hook():
    """Register the NTFF profile hook the boot path skips when the stub
    antenv package lacks axon_hooks. Needed only for trace=True runs."""
    if "antenv.axon_hooks" in sys.modules:
        return
    try:
        import antenv
        from trn_agent_boot.trn_boot import _ntff_profile_via_ctypes
    except ImportError:
        return
    mod = types.ModuleType("antenv.axon_hooks")
    store = [None]
    mod.set_axon_ntff_profile_hook = lambda h: store.__setitem__(0, h)
    mod.get_axon_ntff_profile_hook = lambda: store[0]
    sys.modules["antenv.axon_hooks"] = mod
    antenv.axon_hooks = mod
    hook = _ntff_profile_via_ctypes("/opt/axon/libaxon_pjrt.so")
    mod.set_axon_ntff_profile_hook(hook)


def _emit(tc, aps):
    nc = tc.nc
    table = aps["table"]
    ltab = aps["ltab"]
    offs = aps["offs"]
    mask0 = aps["mask0"]
    mask1 = aps["mask1"]
    thmix0 = aps["thmix0"]
    thmix1 = aps["thmix1"]
    th2p = aps["th2p"]
    out = aps["out"]

    from contextlib import ExitStack
    with ExitStack() as ctx:
        const = ctx.enter_context(tc.tile_pool(name="const", bufs=1))
        gpool = ctx.enter_context(tc.tile_pool(name="gath", bufs=4))
        spool = ctx.enter_context(tc.tile_pool(name="scaled", bufs=4))
        lpool = ctx.enter_context(tc.tile_pool(name="lhst", bufs=4))
        fpool = ctx.enter_context(tc.tile_pool(name="final", bufs=4))
        ppool = ctx.enter_context(tc.tile_pool(name="psum", bufs=4, space="PSUM"))
        dpool = ctx.enter_context(tc.tile_pool(name="dram", bufs=1, space="DRAM"))

        # ---- constants ----
        offs_sb = const.tile([128, NT], I32)
        nc.sync.dma_start(offs_sb[:], offs[:])
        m0_sb = const.tile([126, 112], F32)
        nc.sync.dma_start(m0_sb[:], mask0[:])
        m1_sb = const.tile([126, 112], F32)
        nc.sync.dma_start(m1_sb[:], mask1[:])
        tm0_sb = const.tile([128, R], F32)
        nc.sync.dma_start(tm0_sb[:], thmix0[:])
        tm1_sb = const.tile([128, R], F32)
        nc.sync.dma_start(tm1_sb[:], thmix1[:])
        th2_sb = const.tile([128, R], F32)
        nc.sync.dma_start(th2_sb[:], th2p[:])

        # ---- T2 = mean_n theta2*kron via 8 chunk matmuls + AllReduce ----
        psum_t2 = ppool.tile([M, R], F32, space="PSUM")
        for j in range(NL // 128):
            ch = gpool.tile([128, TW], F32, tag="gath")
            nc.sync.dma_start(ch[:], ltab[j * 128:(j + 1) * 128, :])
            v2c = spool.tile([128, R], F32, tag="scaled")
            nc.vector.tensor_mul(v2c[:], ch[:, :R], th2_sb[:])
            nc.tensor.matmul(
                psum_t2[:], lhsT=ch[:, R:TW], rhs=v2c[:],
                start=(j == 0), stop=(j == NL // 128 - 1),
            )
        t2_sb = const.tile([M, R], F32)
        nc.vector.tensor_copy(t2_sb[:], psum_t2[:])
        cc_in = dpool.tile([M, R], F32)
        cc_out = dpool.tile([M, R], F32)
        nc.gpsimd.dma_start(cc_in[:], t2_sb[:])
        nc.gpsimd.collective_compute(
            "AllReduce",
            mybir.AluOpType.add,
            replica_groups=[list(range(NCORES))],
            ins=[cc_in.opt()],
            outs=[cc_out.opt()],
        )
        # broadcast (8,256) -> (112,256): partition p = 8*b + i holds T2[i,:]
        t2b_sb = const.tile([112, R], F32)
        nc.sync.dma_start(
            t2b_sb[:], cc_out[None, :, :].broadcast_to((BT, M, R)))

        # ---- main loop over node tiles ----
        for t in range(NT):
            bcnt = BT if t < NFULL else REM
            edg = EPN * bcnt
            msk = m0_sb if t < NFULL else m1_sb
            tmx = tm0_sb if t < NFULL else tm1_sb

            g = gpool.tile([128, TW], F32, tag="gath")
            nc.gpsimd.indirect_dma_start(
                out=g[:edg, :],
                out_offset=None,
                in_=table[:],
                in_offset=IndirectOffsetOnAxis(ap=offs_sb[:edg, t:t + 1], axis=0),
            )
            gs = spool.tile([128, R], F32, tag="scaled")
            # per-row theta pattern: self rows theta1, neighbor rows theta3/K
            nc.vector.tensor_mul(gs[:edg], g[:edg, :R], tmx[:edg])

            # block-diagonal lhsT[p, 8b+i] = mask[p, 8b+i] * x_gathered[p, i]
            lhsT = lpool.tile([128, 112], F32)
            nc.vector.tensor_mul(
                lhsT[:edg].rearrange("p (b i) -> p b i", i=M),
                msk[:edg].rearrange("p (b i) -> p b i", i=M),
                g[:edg, R:TW][:, None, :].broadcast_to((edg, BT, M)),
            )

            psum = ppool.tile([112, R], F32, space="PSUM", tag="psum")
            nc.tensor.matmul(
                psum[:], lhsT=lhsT[:edg, :], rhs=gs[:edg, :],
                start=True, stop=True,
            )

            fin = fpool.tile([112, R], F32)
            nc.vector.tensor_add(fin[:], psum[:], t2b_sb[:])

            n0 = t * BT
            nc.sync.dma_start(
                out[n0:n0 + bcnt, :].rearrange("b (i q) -> b i q", q=R),
                fin[:M * bcnt, :],
            )


_PROGRAM = None


def _build_program():
    global _PROGRAM
    if _PROGRAM is not None:
        return _PROGRAM
    nc = bacc.Bacc("TRN2", target_bir_lowering=False, debug=False,
                   enable_asserts=False, num_devices=NCORES)
    aps = {
        "table": nc.dram_tensor("table", (N, TW), F32, kind="ExternalInput").ap(),
        "ltab": nc.dram_tensor("ltab", (NL, TW), F32, kind="ExternalInput").ap(),
        "offs": nc.dram_tensor("offs", (128, NT), I32, kind="ExternalInput").ap(),
        "mask0": nc.dram_tensor("mask0", (126, 112), F32, kind="ExternalInput").ap(),
        "mask1": nc.dram_tensor("mask1", (126, 112), F32, kind="ExternalInput").ap(),
        "thmix0": nc.dram_tensor("thmix0", (128, R), F32, kind="ExternalInput").ap(),
        "thmix1": nc.dram_tensor("thmix1", (128, R), F32, kind="ExternalInput").ap(),
        "th2p": nc.dram_tensor("th2p", (128, R), F32, kind="ExternalInput").ap(),
        "out": nc.dram_tensor("out", (NL, OUTW), F32, kind="ExternalOutput").ap(),
    }
    with tile.TileContext(nc) as tc:
        _emit(tc, aps)
    nc.compile()
    nc.m = get_hw_module(nc.m)
    _PROGRAM = nc
    return nc


def _prep_inputs(x, v, theta1, theta2, theta3, neighbors_indices):
    x = np.asarray(x, dtype=np.float32).reshape(N, M)
    v = np.asarray(v, dtype=np.float32).reshape(N, R)
    theta1 = np.asarray(theta1, dtype=np.float32)
    theta2 = np.asarray(theta2, dtype=np.float32)
    theta3 = np.asarray(theta3, dtype=np.float32)
    nbr = np.asarray(neighbors_indices).astype(np.int32)  # (N, K)

    table = np.ascontiguousarray(np.concatenate([v, x], axis=1))

    th1row = np.tile(theta1, D)
    th3row = np.tile(theta3, D) / K
    th2p = np.ascontiguousarray(
        np.broadcast_to((np.tile(theta2, D) / N)[None, :], (128, R)))

    def make_thmix(bcnt):
        tm = np.zeros((128, R), np.float32)
        tm[:bcnt] = th1row
        tm[bcnt:bcnt + K * bcnt] = th3row
        return tm

    thmix0 = make_thmix(BT)
    thmix1 = make_thmix(REM) if REM else thmix0

    def make_mask(bcnt):
        mk = np.zeros((126, 112), np.float32)
        b = np.arange(bcnt)
        for i in range(M):
            mk[b, M * b + i] = 1.0                      # self rows
        p = bcnt + np.arange(K * bcnt)
        bb = (p - bcnt) // K
        for i in range(M):
            mk[p, M * bb + i] = 1.0                     # neighbor rows
        return mk

    mask0 = make_mask(BT)
    mask1 = make_mask(REM) if REM else mask0

    in_maps = []
    for c in range(NCORES):
        lo = c * NL
        nodes = lo + np.arange(NL)
        offs = np.zeros((128, NT), np.int32)
        nf = nodes[:NFULL * BT].reshape(NFULL, BT)      # (73, 14)
        offs[:BT, :NFULL] = nf.T
        offs[BT:BT + K * BT, :NFULL] = nbr[nf].reshape(NFULL, BT * K).T
        if REM:
            nl_ = nodes[NFULL * BT:]
            offs[:REM, NFULL] = nl_
            offs[REM:REM + K * REM, NFULL] = nbr[nl_].reshape(-1)
        in_maps.append({
            "table": table,
            "ltab": np.ascontiguousarray(table[lo:lo + NL]),
            "offs": offs,
            "mask0": mask0,
            "mask1": mask1,
            "thmix0": thmix0,
            "thmix1": thmix1,
            "th2p": th2p,
        })
    return in_maps


def kernel(x, v, theta1, theta2, theta3, neighbors_indices,
           _trace=False, _trace_kwargs=None):
    nc = _build_program()
    in_maps = _prep_inputs(x, v, theta1, theta2, theta3, neighbors_indices)
    if _trace:
        _ensure_profile_hook()
    res = run_bass_kernel_spmd(
        nc, in_maps, core_ids=list(range(NCORES)), trace=_trace,
        **(_trace_kwargs or {}),
    )
    out = np.concatenate(
        [res.results[c]["out"].reshape(NL, M * D, C) for c in range(NCORES)],
        axis=0,
    )
    if _trace:
        kernel.last_result = res
    return out


# revision 18
# speedup vs baseline: 1.1799x; 1.1799x over previous
"""Trainium2 Bass kernel for nn_KroneckerLayer.

out[n] = theta1 * kron[n] + mean_n(theta2 * kron) + theta3 * mean_k kron[nbr(n,k)]
with kron[n] = outer(x[n] in R^8, v[n] in R^256), output (8192, 64, 32) f32.

Strategy: shard nodes across 8 cores (1024 each). Per 14-node tile, one
indirect-DMA gather pulls the 126 needed table rows (self + 8 neighbors,
row = [v | x] concat, 264 f32) into SBUF; DVE scales rows by theta
patterns, a mask multiply builds a block-diagonal lhsT, and a single PE
matmul emits all 14 nodes' 2048 outputs into PSUM. The global-mean term
is 8 chunk matmuls over the local shard + an 8KB AllReduce, folded into
the PSUM->SBUF eviction add.
"""

import sys
import types

for _p in ("/opt/trn_rl_repo", "/root/.axon_site/_ro/trn_rl_repo"):
    if _p not in sys.path:
        sys.path.insert(0, _p)

import numpy as np

import concourse.bass as bass
import concourse.bacc as bacc
import concourse.tile as tile
import concourse.mybir as mybir
from concourse.bass import IndirectOffsetOnAxis
from concourse.bass_interp import get_hw_module
import concourse.bass_utils as bass_utils
from concourse.bass_utils import run_bass_kernel_spmd

# Avoid S3 artifact upload during tracing (no creds in container)
bass_utils.upload_artifacts = lambda tmpdir: "local://" + tmpdir

N, M, D, C, K = 8192, 8, 8, 32, 8
R = D * C              # 256, v row
TW = R + M             # 264, table row [v | x]
NCORES = 8
NL = N // NCORES       # 1024 local nodes per core
BT = 14                # nodes per tile
EPN = K + 1            # 9 edges per node (self + K neighbors)
NFULL = NL // BT       # 73 full tiles
REM = NL - NFULL * BT  # 2 nodes in the last tile
NT = NFULL + (1 if REM else 0)  # 74
OUTW = M * R           # 2048 floats per node

F32 = mybir.dt.float32
F32R = mybir.dt.float32r
I32 = mybir.dt.int32

USE_F32R = True  # PE fp32r: 4x matmul rate, slightly reduced multiply precision
MM_DT = F32R if USE_F32R else F32


def _ensure_profile_hook():
    """Register the NTFF profile hook the boot path skips when the stub
    antenv package lacks axon_hooks. Needed only for trace=True runs."""
    if "antenv.axon_hooks" in sys.modules:
        return
    try:
        import antenv
        from trn_agent_boot.trn_boot import _ntff_profile_via_ctypes
    except ImportError:
        return
    mod = types.ModuleType("antenv.axon_hooks")
    store = [None]
    mod.set_axon_ntff_profile_hook = lambda h: store.__setitem__(0, h)
    mod.get_axon_ntff_profile_hook = lambda: store[0]
    sys.modules["antenv.axon_hooks"] = mod
    antenv.axon_hooks = mod
    hook = _ntff_profile_via_ctypes("/opt/axon/libaxon_pjrt.so")
    mod.set_axon_ntff_profile_hook(hook)


def _emit(tc, aps):
    nc = tc.nc
    table = aps["table"]
    ltab = aps["ltab"]
    offs = aps["offs"]
    mask0 = aps["mask0"]
    mask1 = aps["mask1"]
    thmix0 = aps["thmix0"]
    thmix1 = aps["thmix1"]
    th2p = aps["th2p"]
    out = aps["out"]

    from contextlib import ExitStack
    with ExitStack() as ctx:
        const = ctx.enter_context(tc.tile_pool(name="const", bufs=1))
        gpool = ctx.enter_context(tc.tile_pool(name="gath", bufs=6))
        cpool = ctx.enter_context(tc.tile_pool(name="chunk", bufs=4))
        spool = ctx.enter_context(tc.tile_pool(name="scaled", bufs=6))
        lpool = ctx.enter_context(tc.tile_pool(name="lhst", bufs=6))
        fpool = ctx.enter_context(tc.tile_pool(name="final", bufs=6))
        ppool = ctx.enter_context(tc.tile_pool(name="psum", bufs=6, space="PSUM"))
        dpool = ctx.enter_context(tc.tile_pool(name="dram", bufs=1, space="DRAM"))

        # ---- constants ----
        offs_sb = const.tile([128, NT], I32)
        nc.sync.dma_start(offs_sb[:], offs[:])
        m0_sb = const.tile([126, 112], F32)
        nc.sync.dma_start(m0_sb[:], mask0[:])
        m1_sb = const.tile([126, 112], F32)
        nc.sync.dma_start(m1_sb[:], mask1[:])
        tm0_sb = const.tile([128, R], F32)
        nc.sync.dma_start(tm0_sb[:], thmix0[:])
        tm1_sb = const.tile([128, R], F32)
        nc.sync.dma_start(tm1_sb[:], thmix1[:])
        th2_sb = const.tile([M, R], F32)
        nc.sync.dma_start(th2_sb[:], th2p[:])

        # ---- T2 = mean_n theta2*kron, computed redundantly on every core.
        # theta2 factors out of the contraction: T2 = th2p * (X^T V) / N
        NCH = N // 128
        # all x columns, one partition-strided DMA from the table, then one
        # rounding pass: xfull_r[p, 8j+i] = x[128j+p, i] in matmul dtype
        xfull = const.tile([128, NCH * M], F32)
        nc.sync.dma_start(
            xfull[:],
            table[:, R:TW].rearrange("(j p) i -> p j i", p=128),
        )
        xfull_r = const.tile([128, NCH * M], MM_DT)
        nc.scalar.copy(xfull_r[:], xfull[:])

        psum_t2 = ppool.tile([M, R], F32, space="PSUM", tag="psumt2", bufs=1)
        for j in range(NCH):
            ch = cpool.tile([128, R], F32, tag="chunk")
            nc.sync.dma_start(ch[:], table[j * 128:(j + 1) * 128, :R])
            chr_ = cpool.tile([128, R], MM_DT, tag="chunkr")
            nc.scalar.copy(chr_[:], ch[:])
            nc.tensor.matmul(
                psum_t2[:],
                lhsT=xfull_r[:, j * M:(j + 1) * M],
                rhs=chr_[:],
                start=(j == 0), stop=(j == NCH - 1),
            )
        t2_sb = const.tile([M, R], F32)
        nc.vector.tensor_mul(t2_sb[:], psum_t2[:], th2_sb[:])
        t2_dram = dpool.tile([M, R], F32)
        nc.sync.dma_start(t2_dram[:], t2_sb[:])
        # broadcast (8,256) -> (112,256): partition p = 8*b + i holds T2[i,:]
        t2b_sb = const.tile([112, R], F32)
        nc.sync.dma_start(
            t2b_sb[:], t2_dram[None, :, :].broadcast_to((BT, M, R)))

        # ---- main loop over node tiles ----
        for t in range(NT):
            bcnt = BT if t < NFULL else REM
            edg = EPN * bcnt
            msk = m0_sb if t < NFULL else m1_sb
            tmx = tm0_sb if t < NFULL else tm1_sb
            n0 = t * BT

            g = gpool.tile([128, TW], F32, tag="gath")
            # self rows: direct DMA from the local shard (contiguous)
            nc.sync.dma_start(g[:bcnt, :], ltab[n0:n0 + bcnt, :])
            # neighbor rows: indirect gather from the full table
            nc.gpsimd.indirect_dma_start(
                out=g[bcnt:edg, :],
                out_offset=None,
                in_=table[:],
                in_offset=IndirectOffsetOnAxis(
                    ap=offs_sb[:K * bcnt, t:t + 1], axis=0),
            )
            gs = spool.tile([128, R], MM_DT, tag="scaled")
            # per-row theta pattern: self rows theta1, neighbor rows theta3/K
            nc.vector.tensor_mul(gs[:edg], g[:edg, :R], tmx[:edg])

            # block-diagonal lhsT[p, 8b+i] = mask[p, 8b+i] * x_gathered[p, i]
            lhsT = lpool.tile([128, 112], MM_DT)
            nc.vector.tensor_mul(
                lhsT[:edg].rearrange("p (b i) -> p b i", i=M),
                msk[:edg].rearrange("p (b i) -> p b i", i=M),
                g[:edg, R:TW][:, None, :].broadcast_to((edg, BT, M)),
            )

            psum = ppool.tile([112, R], F32, space="PSUM", tag="psum")
            nc.tensor.matmul(
                psum[:], lhsT=lhsT[:edg, :], rhs=gs[:edg, :],
                start=True, stop=True,
            )

            fin = fpool.tile([112, R], F32)
            nc.vector.tensor_add(fin[:], psum[:], t2b_sb[:])

            nc.scalar.dma_start(
                out[n0:n0 + bcnt, :].rearrange("b (i q) -> b i q", q=R),
                fin[:M * bcnt, :],
            )


_PROGRAM = None


def _build_program():
    global _PROGRAM
    if _PROGRAM is not None:
        return _PROGRAM
    nc = bacc.Bacc("TRN2", target_bir_lowering=False, debug=False,
                   enable_asserts=False, num_devices=NCORES)
    aps = {
        "table": nc.dram_tensor("table", (N, TW), F32, kind="ExternalInput").ap(),
        "ltab": nc.dram_tensor("ltab", (NL, TW), F32, kind="ExternalInput").ap(),
        "offs": nc.dram_tensor("offs", (128, NT), I32, kind="ExternalInput").ap(),
        "mask0": nc.dram_tensor("mask0", (126, 112), F32, kind="ExternalInput").ap(),
        "mask1": nc.dram_tensor("mask1", (126, 112), F32, kind="ExternalInput").ap(),
        "thmix0": nc.dram_tensor("thmix0", (128, R), F32, kind="ExternalInput").ap(),
        "thmix1": nc.dram_tensor("thmix1", (128, R), F32, kind="ExternalInput").ap(),
        "th2p": nc.dram_tensor("th2p", (M, R), F32, kind="ExternalInput").ap(),
        "out": nc.dram_tensor("out", (NL, OUTW), F32, kind="ExternalOutput").ap(),
    }
    with tile.TileContext(nc) as tc:
        _emit(tc, aps)
    nc.compile()
    nc.m = get_hw_module(nc.m)
    _PROGRAM = nc
    return nc


def _prep_inputs(x, v, theta1, theta2, theta3, neighbors_indices):
    x = np.asarray(x, dtype=np.float32).reshape(N, M)
    v = np.asarray(v, dtype=np.float32).reshape(N, R)
    theta1 = np.asarray(theta1, dtype=np.float32)
    theta2 = np.asarray(theta2, dtype=np.float32)
    theta3 = np.asarray(theta3, dtype=np.float32)
    nbr = np.asarray(neighbors_indices).astype(np.int32)  # (N, K)

    table = np.ascontiguousarray(np.concatenate([v, x], axis=1))

    th1row = np.tile(theta1, D)
    th3row = np.tile(theta3, D) / K
    th2p = np.ascontiguousarray(
        np.broadcast_to((np.tile(theta2, D) / N)[None, :], (M, R)))

    def make_thmix(bcnt):
        tm = np.zeros((128, R), np.float32)
        tm[:bcnt] = th1row
        tm[bcnt:bcnt + K * bcnt] = th3row
        return tm

    thmix0 = make_thmix(BT)
    thmix1 = make_thmix(REM) if REM else thmix0

    def make_mask(bcnt):
        mk = np.zeros((126, 112), np.float32)
        b = np.arange(bcnt)
        for i in range(M):
            mk[b, M * b + i] = 1.0                      # self rows
        p = bcnt + np.arange(K * bcnt)
        bb = (p - bcnt) // K
        for i in range(M):
            mk[p, M * bb + i] = 1.0                     # neighbor rows
        return mk

    mask0 = make_mask(BT)
    mask1 = make_mask(REM) if REM else mask0

    in_maps = []
    for c in range(NCORES):
        lo = c * NL
        nodes = lo + np.arange(NL)
        # neighbor-only row indices; self rows come via direct DMA from ltab
        offs = np.zeros((128, NT), np.int32)
        nf = nodes[:NFULL * BT].reshape(NFULL, BT)      # (73, 14)
        offs[:K * BT, :NFULL] = nbr[nf].reshape(NFULL, BT * K).T
        if REM:
            nl_ = nodes[NFULL * BT:]
            offs[:K * REM, NFULL] = nbr[nl_].reshape(-1)
        in_maps.append({
            "table": table,
            "ltab": np.ascontiguousarray(table[lo:lo + NL]),
            "offs": offs,
            "mask0": mask0,
            "mask1": mask1,
            "thmix0": thmix0,
            "thmix1": thmix1,
            "th2p": th2p,
        })
    return in_maps


def kernel(x, v, theta1, theta2, theta3, neighbors_indices,
           _trace=False, _trace_kwargs=None):
    nc = _build_program()
    in_maps = _prep_inputs(x, v, theta1, theta2, theta3, neighbors_indices)
    if _trace:
        _ensure_profile_hook()
    res = run_bass_kernel_spmd(
        nc, in_maps, core_ids=list(range(NCORES)), trace=_trace,
        **(_trace_kwargs or {}),
    )
    out = np.concatenate(
        [res.results[c]["out"].reshape(NL, M * D, C) for c in range(NCORES)],
        axis=0,
    )
    if _trace:
        kernel.last_result = res
    return out


# revision 25
# speedup vs baseline: 1.4843x; 1.2580x over previous
"""Trainium2 Bass kernel for nn_KroneckerLayer.

out[n] = theta1 * kron[n] + mean_n(theta2 * kron) + theta3 * mean_k kron[nbr(n,k)]
with kron[n] = outer(x[n] in R^8, v[n] in R^256), output (8192, 64, 32) f32.

Strategy: shard nodes across 8 cores (1024 each). Per 14-node tile, one
indirect-DMA gather pulls the 126 needed table rows (self + 8 neighbors,
row = [v | x] concat, 264 f32) into SBUF; DVE scales rows by theta
patterns, a mask multiply builds a block-diagonal lhsT, and a single PE
matmul emits all 14 nodes' 2048 outputs into PSUM. The global-mean term
is 8 chunk matmuls over the local shard + an 8KB AllReduce, folded into
the PSUM->SBUF eviction add.
"""

import sys
import types

for _p in ("/opt/trn_rl_repo", "/root/.axon_site/_ro/trn_rl_repo"):
    if _p not in sys.path:
        sys.path.insert(0, _p)

import numpy as np

import concourse.bass as bass
import concourse.bacc as bacc
import concourse.tile as tile
import concourse.mybir as mybir
from concourse.bass import IndirectOffsetOnAxis
from concourse.bass_interp import get_hw_module
import concourse.bass_utils as bass_utils
from concourse.bass_utils import run_bass_kernel_spmd

# Avoid S3 artifact upload during tracing (no creds in container)
bass_utils.upload_artifacts = lambda tmpdir: "local://" + tmpdir

N, M, D, C, K = 8192, 8, 8, 32, 8
R = D * C              # 256, v row
TW = R + M             # 264, table row [v | x]
NCORES = 8
NL = N // NCORES       # 1024 local nodes per core
BT = 13                # nodes per tile
NFULL = NL // BT       # 78 full tiles
REM = NL - NFULL * BT  # 10 nodes in the last tile
NT = NFULL + (1 if REM else 0)  # 79
OUTW = M * R           # 2048 floats per node
GW = 4                 # output tiles merged per store DMA
# g-tile row layout: [0:8] T2-inject rows, [8:8+8b] neighbors, then b self

F32 = mybir.dt.float32
F32R = mybir.dt.float32r
I32 = mybir.dt.int32

USE_F32R = True  # PE fp32r: 4x matmul rate, slightly reduced multiply precision
MM_DT = F32R if USE_F32R else F32


def _ensure_profile_hook():
    """Register the NTFF profile hook the boot path skips when the stub
    antenv package lacks axon_hooks. Needed only for trace=True runs."""
    if "antenv.axon_hooks" in sys.modules:
        return
    try:
        import antenv
        from trn_agent_boot.trn_boot import _ntff_profile_via_ctypes
    except ImportError:
        return
    mod = types.ModuleType("antenv.axon_hooks")
    store = [None]
    mod.set_axon_ntff_profile_hook = lambda h: store.__setitem__(0, h)
    mod.get_axon_ntff_profile_hook = lambda: store[0]
    sys.modules["antenv.axon_hooks"] = mod
    antenv.axon_hooks = mod
    hook = _ntff_profile_via_ctypes("/opt/axon/libaxon_pjrt.so")
    mod.set_axon_ntff_profile_hook(hook)


def _emit(tc, aps):
    nc = tc.nc
    table = aps["table"]
    ltab = aps["ltab"]
    offs = aps["offs"]
    mask0 = aps["mask0"]
    mask1 = aps["mask1"]
    thmix0 = aps["thmix0"]
    thmix1 = aps["thmix1"]
    th2p = aps["th2p"]
    out = aps["out"]

    vtab_bf = aps["vtab_bf"]
    BF16 = mybir.dt.bfloat16

    from contextlib import ExitStack
    with ExitStack() as ctx:
        const = ctx.enter_context(tc.tile_pool(name="const", bufs=1))
        gpool = ctx.enter_context(tc.tile_pool(name="gath", bufs=16))
        cpool = ctx.enter_context(tc.tile_pool(name="chunk", bufs=2))
        spool = ctx.enter_context(tc.tile_pool(name="scaled", bufs=8))
        lpool = ctx.enter_context(tc.tile_pool(name="lhst", bufs=8))
        fpool = ctx.enter_context(tc.tile_pool(name="final", bufs=4))
        ppool = ctx.enter_context(tc.tile_pool(name="psum", bufs=6, space="PSUM"))

        # ---- constants ----
        offs_sb = const.tile([128, NT], I32)
        nc.sync.dma_start(offs_sb[:], offs[:])
        m0_sb = const.tile([125, M * BT], F32)
        nc.sync.dma_start(m0_sb[:], mask0[:])
        m1_sb = const.tile([M + 9 * REM, M * REM], F32)
        nc.sync.dma_start(m1_sb[:], mask1[:])
        tm0_sb = const.tile([128, R], F32)
        nc.sync.dma_start(tm0_sb[:], thmix0[:])
        tm1_sb = const.tile([128, R], F32)
        nc.sync.dma_start(tm1_sb[:], thmix1[:])
        th2_sb = const.tile([M, R], F32)
        nc.sync.dma_start(th2_sb[:], th2p[:])

        # ---- T2 = mean_n theta2*kron, computed redundantly on every core.
        # theta2 factors out of the contraction: T2 = th2p * (X^T V) / N.
        # The mean is precision-insensitive (output magnitude ~sigma/sqrt(N)),
        # so the v stream is bf16.
        NCH = N // 128
        # all x columns, one partition-strided DMA from the table, then one
        # rounding pass: xfull[p, 8j+i] = x[128j+p, i]
        xfull = const.tile([128, NCH * M], F32)
        nc.sync.dma_start(
            xfull[:],
            table[:, R:TW].rearrange("(j p) i -> p j i", p=128),
        )
        xfull_bf = const.tile([128, NCH * M], BF16)
        nc.scalar.copy(xfull_bf[:], xfull[:])

        psum_t2 = ppool.tile([M, R], F32, space="PSUM", tag="psumt2", bufs=1)
        JPER = 8  # chunks per mega-DMA
        for jg in range(NCH // JPER):
            ch = cpool.tile([128, JPER * R], BF16, tag="chunk")
            nc.sync.dma_start(
                ch[:].rearrange("p (j e) -> p j e", j=JPER),
                vtab_bf[jg * JPER * 128:(jg + 1) * JPER * 128, :]
                .rearrange("(j p) e -> p j e", p=128),
            )
            for j in range(JPER):
                jj = jg * JPER + j
                nc.tensor.matmul(
                    psum_t2[:],
                    lhsT=xfull_bf[:, jj * M:(jj + 1) * M],
                    rhs=ch[:, j * R:(j + 1) * R],
                    start=(jj == 0), stop=(jj == NCH - 1),
                )
        # t2ext rows: [v-part] = T2 values, [x-part] = 1.0 so the lhsT
        # mask-multiply leaves the constant identity block intact
        t2e_sb = const.tile([M, TW], F32)
        nc.vector.tensor_mul(t2e_sb[:, :R], psum_t2[:], th2_sb[:])
        nc.vector.memset(t2e_sb[:, R:TW], 1.0)

        # ---- main loop over node tiles ----
        fin = None
        for t in range(NT):
            bcnt = BT if t < NFULL else REM
            edg = M + (K + 1) * bcnt     # 8 T2 rows + 8b nbr + b self
            nbr0, nbr1 = M, M + K * bcnt
            msk = m0_sb if t < NFULL else m1_sb
            tmx = tm0_sb if t < NFULL else tm1_sb
            n0 = t * BT

            g = gpool.tile([128, TW], F32, tag="gath")
            # T2-inject rows
            nc.vector.tensor_copy(g[:M, :], t2e_sb[:])
            # neighbor rows: indirect gather from the full table
            nc.gpsimd.indirect_dma_start(
                out=g[nbr0:nbr1, :],
                out_offset=None,
                in_=table[:],
                in_offset=IndirectOffsetOnAxis(
                    ap=offs_sb[:K * bcnt, t:t + 1], axis=0),
            )
            # self rows: direct DMA from the local shard (contiguous)
            nc.sync.dma_start(g[nbr1:edg, :], ltab[n0:n0 + bcnt, :])

            gs = spool.tile([128, R], MM_DT, tag="scaled")
            # per-row theta: T2 rows 1.0, neighbor rows theta3/K, self theta1
            nc.vector.tensor_mul(gs[:edg], g[:edg, :R], tmx[:edg])

            # block-diagonal lhsT[p, 8b+i] = mask[p, 8b+i] * x_row[p, i]
            lhsT = lpool.tile([128, M * BT], MM_DT)
            nc.vector.tensor_mul(
                lhsT[:edg, :M * bcnt].rearrange("p (b i) -> p b i", i=M),
                msk[:edg, :M * bcnt].rearrange("p (b i) -> p b i", i=M),
                g[:edg, R:TW][:, None, :].broadcast_to((edg, bcnt, M)),
            )

            psum = ppool.tile([M * BT, R], F32, space="PSUM", tag="psum")
            nc.tensor.matmul(
                psum[:M * bcnt, :], lhsT=lhsT[:edg, :M * bcnt],
                rhs=gs[:edg, :], start=True, stop=True,
            )

            # evict on the scalar engine into the merged store tile
            gslot = t % GW
            if gslot == 0:
                fin = fpool.tile([M * BT, GW * R], F32)
            nc.scalar.copy(
                fin[:M * bcnt, gslot * R:(gslot + 1) * R], psum[:M * bcnt, :])

            if gslot == GW - 1 or t == NT - 1:
                t0 = t - gslot
                nfull_g = gslot + 1 - (1 if bcnt != BT else 0)
                if nfull_g > 0:
                    nc.scalar.dma_start(
                        out[t0 * BT:(t0 + nfull_g) * BT, :]
                        .rearrange("(g b) (i q) -> b i g q", g=nfull_g, q=R),
                        fin[:M * BT, :nfull_g * R]
                        .rearrange("p (g q) -> p g q", g=nfull_g),
                    )
                if bcnt != BT:
                    nc.scalar.dma_start(
                        out[NFULL * BT:NL, :].rearrange("b (i q) -> b i q", q=R),
                        fin[:M * REM, gslot * R:(gslot + 1) * R],
                    )


_PROGRAM = None


def _build_program():
    global _PROGRAM
    if _PROGRAM is not None:
        return _PROGRAM
    nc = bacc.Bacc("TRN2", target_bir_lowering=False, debug=False,
                   enable_asserts=False, num_devices=NCORES)
    BF16 = mybir.dt.bfloat16
    aps = {
        "table": nc.dram_tensor("table", (N, TW), F32, kind="ExternalInput").ap(),
        "vtab_bf": nc.dram_tensor("vtab_bf", (N, R), BF16, kind="ExternalInput").ap(),
        "ltab": nc.dram_tensor("ltab", (NL, TW), F32, kind="ExternalInput").ap(),
        "offs": nc.dram_tensor("offs", (128, NT), I32, kind="ExternalInput").ap(),
        "mask0": nc.dram_tensor("mask0", (125, M * BT), F32, kind="ExternalInput").ap(),
        "mask1": nc.dram_tensor("mask1", (M + 9 * REM, M * REM), F32, kind="ExternalInput").ap(),
        "thmix0": nc.dram_tensor("thmix0", (128, R), F32, kind="ExternalInput").ap(),
        "thmix1": nc.dram_tensor("thmix1", (128, R), F32, kind="ExternalInput").ap(),
        "th2p": nc.dram_tensor("th2p", (M, R), F32, kind="ExternalInput").ap(),
        "out": nc.dram_tensor("out", (NL, OUTW), F32, kind="ExternalOutput").ap(),
    }
    with tile.TileContext(nc) as tc:
        _emit(tc, aps)
    nc.compile()
    nc.m = get_hw_module(nc.m)
    _PROGRAM = nc
    return nc


def _prep_inputs(x, v, theta1, theta2, theta3, neighbors_indices):
    x = np.asarray(x, dtype=np.float32).reshape(N, M)
    v = np.asarray(v, dtype=np.float32).reshape(N, R)
    theta1 = np.asarray(theta1, dtype=np.float32)
    theta2 = np.asarray(theta2, dtype=np.float32)
    theta3 = np.asarray(theta3, dtype=np.float32)
    nbr = np.asarray(neighbors_indices).astype(np.int32)  # (N, K)

    table = np.ascontiguousarray(np.concatenate([v, x], axis=1))
    bf16 = mybir.dt.np(mybir.dt.bfloat16)
    vtab_bf = np.ascontiguousarray(v.astype(bf16))

    th1row = np.tile(theta1, D)
    th3row = np.tile(theta3, D) / K
    th2p = np.ascontiguousarray(
        np.broadcast_to((np.tile(theta2, D) / N)[None, :], (M, R)))

    def make_thmix(bcnt):
        tm = np.zeros((128, R), np.float32)
        tm[:M] = 1.0                                     # T2-inject rows
        tm[M:M + K * bcnt] = th3row                      # neighbor rows
        tm[M + K * bcnt:M + (K + 1) * bcnt] = th1row     # self rows
        return tm

    thmix0 = make_thmix(BT)
    thmix1 = make_thmix(REM) if REM else thmix0

    def make_mask(bcnt):
        mk = np.zeros((M + (K + 1) * bcnt, M * bcnt), np.float32)
        for i in range(M):                               # T2 identity rows
            mk[i, i::M] = 1.0
        p = M + np.arange(K * bcnt)                      # neighbor rows
        bb = (p - M) // K
        for i in range(M):
            mk[p, M * bb + i] = 1.0
        b = np.arange(bcnt)                              # self rows
        p = M + K * bcnt + b
        for i in range(M):
            mk[p, M * b + i] = 1.0
        return mk

    mask0 = make_mask(BT)
    mask1 = make_mask(REM) if REM else mask0

    in_maps = []
    for c in range(NCORES):
        lo = c * NL
        nodes = lo + np.arange(NL)
        # neighbor-only row indices; self rows come via direct DMA from ltab
        offs = np.zeros((128, NT), np.int32)
        nf = nodes[:NFULL * BT].reshape(NFULL, BT)      # (73, 14)
        offs[:K * BT, :NFULL] = nbr[nf].reshape(NFULL, BT * K).T
        if REM:
            nl_ = nodes[NFULL * BT:]
            offs[:K * REM, NFULL] = nbr[nl_].reshape(-1)
        in_maps.append({
            "table": table,
            "vtab_bf": vtab_bf,
            "ltab": np.ascontiguousarray(table[lo:lo + NL]),
            "offs": offs,
            "mask0": mask0,
            "mask1": mask1,
            "thmix0": thmix0,
            "thmix1": thmix1,
            "th2p": th2p,
        })
    return in_maps


def kernel(x, v, theta1, theta2, theta3, neighbors_indices,
           _trace=False, _trace_kwargs=None):
    nc = _build_program()
    in_maps = _prep_inputs(x, v, theta1, theta2, theta3, neighbors_indices)
    if _trace:
        _ensure_profile_hook()
    res = run_bass_kernel_spmd(
        nc, in_maps, core_ids=list(range(NCORES)), trace=_trace,
        **(_trace_kwargs or {}),
    )
    out = np.concatenate(
        [res.results[c]["out"].reshape(NL, M * D, C) for c in range(NCORES)],
        axis=0,
    )
    if _trace:
        kernel.last_result = res
    return out
